# revision 61
# baseline (speedup 1.0000x reference)
"""Trainium2 Bass kernel for nn_NormalizingFlow (dense_mlp, 8 flows, H=64).

Math (validated against the jax reference to ~7e-7 rel err):
  Because SPLIT=1, z[:,0] is invariant across all 8 coupling flows, so every
  flow's MLP reads the same scalar z0 = m0 + Lc00*e0.  The affine flow
  composition collapses:
    s_f = tanh(p_f[:2]),  t_f = p_f[2:]          (p_f = MLP_f(z0))
    z2_final = z2_init * exp(sum_f s_f) + sum_f t_f * exp(sum_{g>f} s_g)
    ld       = sum_{f,c} s_{f,c}
  Outputs: z = [z0, exp(z1p), exp(z2p)], ld_total = ld + sum_all(z1p)+sum_all(z2p).

Layout strategy (per core, data-parallel over 8 cores):
  * feature-major MLP: activations [feat_part, sample_free], 2 flows packed
    per 128-partition group (4 groups), block-diag W2 in one K=128 matmul.
  * per 512-sample supertile: L1 (K=1), relu(DVE,+bias), L2 (K=128),
    relu(ACT,+bias), L3 (K=128,M=8) + z-init (K=3,M=3) -> P psum [35,512],
    bias-add copy (ACT) -> SBUF, PE-transpose per 128-chunk -> sample-major.
  * per 8-supertile batch: tanh/suffix-sum/exp/combine in sample-major
    [128, 32chunk, ...] tiles (full-lane DVE/ACT, few instructions).
"""

import numpy as np

N = 1048576
DIM = 3
H = 64
NF = 8
NCORES = 8
NC_SAMP = N // NCORES        # 131072 samples per core
NF_TILE = 512                # samples per supertile (PSUM fp32 bank limit)
CHUNKS = NF_TILE // 128      # 4

_CACHE = {}

# fp32 constant pack (identity for PE transposes + engine biases)
OFF_ID = 0              # [128, 128] identity
OFF_B1 = 128            # [128, 4] relu1 bias
OFF_B2 = 132            # [128, 4] relu2 bias
OFF_B3 = 136            # [32, 1] p bias (b3)
OFF_LC = 137            # [128, 9] replicated [Lc00,m0,Lc10,m1,Lc11,Lc20,m2,Lc21,Lc22]
WPACK_COLS = 146
# fp16 matmul-weight pack
OFF_L2 = 0              # 4 x [128, 128] L2 block-diag lhsT
OFF_L3 = 512            # 4 x [128, 32] L3 lhsT (col-tiled, M=32 strips)
OFF_L1 = 652            # 4 x [2, 128] L1 lhsT (partitions 0-1: [w1; w1])
OFF_LZ = 1164           # [32, 32] fp16 identity for the P transposes
WPACK16_COLS = 1200


def _pack_weights(L_tril, base_mean, W1, b1, W2, b2, W3, b3):
    """Host-side prep of the tiny parameter tensors (fp32)."""
    f32 = np.float32
    L = np.tril(L_tril).astype(f32) + np.eye(DIM, dtype=f32) * f32(1e-6)
    cov = (L @ L.T).astype(f32)
    Lc = np.linalg.cholesky(cov.astype(np.float64)).astype(f32)

    W1c = W1[:, :, 0].astype(f32)                      # [8, 64]
    W1p = (W1c * Lc[0, 0]).astype(f32)
    b1p = (b1 + W1c * base_mean[0]).astype(f32)

    f16 = np.float16
    wpack = np.zeros((128, WPACK_COLS), f32)
    wpk16 = np.zeros((128, WPACK16_COLS), f16)
    for g in range(4):
        fa, fb = 2 * g, 2 * g + 1
        # L2 block-diag lhsT: [:, g*128:(g+1)*128]
        wpk16[:64, OFF_L2 + 128 * g : OFF_L2 + 128 * g + 64] = W2[fa].T
        wpk16[64:, OFF_L2 + 128 * g + 64 : OFF_L2 + 128 * (g + 1)] = W2[fb].T
        # L1 lhsT [2, 128] (rhs rows are [e0_hi; e0_lo])
        wrow = np.concatenate([W1p[fa], W1p[fb]]).astype(f16)
        wpk16[0, OFF_L1 + 128 * g : OFF_L1 + 128 * (g + 1)] = wrow
        wpk16[1, OFF_L1 + 128 * g : OFF_L1 + 128 * (g + 1)] = wrow
        # L3 lhsT [128, 32] per group: filled cols 8g+j (P row 8g+j,
        # j = 2m+c for s, 4+2m+c for t, flow 2g+m); all 4 groups form one
        # M=32 PSUM accumulation group (zeros elsewhere).
        o = OFF_L3 + 32 * g + 8 * g
        wpk16[:64, o + 0] = W3[fa][0]
        wpk16[:64, o + 1] = W3[fa][1]
        wpk16[64:, o + 2] = W3[fb][0]
        wpk16[64:, o + 3] = W3[fb][1]
        wpk16[:64, o + 4] = W3[fa][2]
        wpk16[:64, o + 5] = W3[fa][3]
        wpk16[64:, o + 6] = W3[fb][2]
        wpk16[64:, o + 7] = W3[fb][3]
        wpack[:64, OFF_B1 + g] = b1p[fa]
        wpack[64:, OFF_B1 + g] = b1p[fb]
        wpack[:64, OFF_B2 + g] = b2[fa]
        wpack[64:, OFF_B2 + g] = b2[fb]
        wpack[8 * g + 0, OFF_B3] = b3[fa][0]
        wpack[8 * g + 1, OFF_B3] = b3[fa][1]
        wpack[8 * g + 2, OFF_B3] = b3[fb][0]
        wpack[8 * g + 3, OFF_B3] = b3[fb][1]
        wpack[8 * g + 4, OFF_B3] = b3[fa][2]
        wpack[8 * g + 5, OFF_B3] = b3[fa][3]
        wpack[8 * g + 6, OFF_B3] = b3[fb][2]
        wpack[8 * g + 7, OFF_B3] = b3[fb][3]
    wpack[:, OFF_ID : OFF_ID + 128] = np.eye(128, dtype=f32)
    wpk16[0:32, OFF_LZ : OFF_LZ + 32] = np.eye(32, dtype=f16)
    # replicated Lc/mean scalars for the gpsimd sample-major z-init
    lcv = [Lc[0, 0], base_mean[0], Lc[1, 0], base_mean[1], Lc[1, 1],
           Lc[2, 0], base_mean[2], Lc[2, 1], Lc[2, 2]]
    wpack[:, OFF_LC : OFF_LC + 9] = np.asarray(lcv, f32)[None, :]
    return {"wpack": wpack, "wpk16": wpk16}


def _build_bass(nc_samp=NC_SAMP, sb=8, legalize=True):
    """Build the Bass program for one core processing nc_samp samples."""
    from contextlib import ExitStack

    import concourse.bass as bass
    import concourse.tile as tile
    from concourse import mybir

    f32 = mybir.dt.float32
    Alu = mybir.AluOpType
    Act = mybir.ActivationFunctionType

    f16 = mybir.dt.float16
    st_total = nc_samp // NF_TILE          # supertiles
    nb = st_total // sb                    # batches
    assert st_total % sb == 0
    C = sb * CHUNKS                        # chunks per batch
    NT = nc_samp // 128                    # total chunks

    nc = bass.Bass(trn_type="TRN2")

    # eps arrives pre-transposed + fp16 hi/lo split (host prep):
    # rows [e0h, e0l, e1h, e1l, e2h, e2l, e0h, e1h, e2h]
    eps_d = nc.dram_tensor("eps", [9, nc_samp], f16, kind="ExternalInput")
    epsm_d = nc.dram_tensor("epsm", [128, NT * 3], f32, kind="ExternalInput")
    wpk_d = nc.dram_tensor("wpack", [128, WPACK_COLS], f32, kind="ExternalInput")
    w16_d = nc.dram_tensor("wpk16", [128, WPACK16_COLS], f16, kind="ExternalInput")

    z_d = nc.dram_tensor("z_out", [NT, 3, 128], f32, kind="ExternalOutput")
    ld_d = nc.dram_tensor("ld_out", [NT, 128], f32, kind="ExternalOutput")
    ps_d = nc.dram_tensor("ps_out", [128, 2], f32, kind="ExternalOutput")

    with ExitStack() as ctx:
        tc = ctx.enter_context(tile.TileContext(nc))
        singles = ctx.enter_context(tc.tile_pool(name="singles", bufs=1))
        inp = ctx.enter_context(tc.tile_pool(name="inp", bufs=3))
        mlp = ctx.enter_context(tc.tile_pool(name="mlp", bufs=3))
        psb = ctx.enter_context(tc.tile_pool(name="psb", bufs=2))
        wp = ctx.enter_context(tc.tile_pool(name="wp", bufs=3))
        cmb = ctx.enter_context(tc.tile_pool(name="cmb", bufs=3))
        h1ps = ctx.enter_context(tc.tile_pool(name="h1ps", bufs=2, space="PSUM"))
        h2ps = ctx.enter_context(tc.tile_pool(name="h2ps", bufs=2, space="PSUM"))
        pps = ctx.enter_context(tc.tile_pool(name="pps", bufs=2, space="PSUM"))
        tps = ctx.enter_context(tc.tile_pool(name="tps", bufs=2, space="PSUM"))

        # ---- constants: one DMA per pack + per-engine absorber ops so no
        # matmul/engine op ever needs more than one sync wait (the walrus
        # here fits a single wait per instruction struct). ----
        wpk = singles.tile([128, WPACK_COLS], f32)
        w16 = singles.tile([128, WPACK16_COLS], f16)
        psacc = singles.tile([128, 2], f32)
        scr_v = singles.tile([128, 18], f32)
        scr_a = singles.tile([128, 14], f32)
        nc.sync.dma_start(out=wpk, in_=wpk_d[:, :])
        nc.sync.dma_start(out=w16, in_=w16_d[:, :])
        # PE absorbers: fp16 standalone LDWEIGHTS over the whole fp16 pack,
        # and a dummy fp32 transpose touching the identity region.
        for c0 in range(0, WPACK16_COLS, 128):
            nc.tensor.ldweights(w16[:, c0 : min(c0 + 128, WPACK16_COLS)])
        dpt = tps.tile([1, 2], f32, tag="tp")
        nc.tensor.transpose(
            dpt[0:1, 0:1], wpk[0:1, OFF_ID : OFF_ID + 1],
            wpk[0:1, OFF_ID : OFF_ID + 1],
        )
        nc.vector.tensor_copy(out=scr_v, in_=wpk[:, OFF_B1 : OFF_LC + 9])
        nc.scalar.copy(out=scr_a, in_=wpk[:, OFF_B2 : OFF_LC + 9])
        nc.vector.memset(psacc, 0.0)

        wL2_s = w16[:, OFF_L2 : OFF_L2 + 512].rearrange("p (g m) -> p g m", m=128)
        wL3_s = w16[:, OFF_L3 : OFF_L3 + 128].rearrange("p (g m) -> p g m", m=32)
        wL1_s = w16[0:2, OFF_L1 : OFF_L1 + 512].rearrange("p (g m) -> p g m", m=128)
        b1_s = wpk[:, OFF_B1 : OFF_B1 + 4]
        b2_s = wpk[:, OFF_B2 : OFF_B2 + 4]
        b3z_s = wpk[0:32, OFF_B3 : OFF_B3 + 1]
        id_s = wpk[:, OFF_ID : OFF_ID + 128]
        id16 = w16[0:32, OFF_LZ : OFF_LZ + 32]
        lcb = wpk[:, OFF_LC : OFF_LC + 9]

        for b in range(nb):
            e8 = inp.tile([9, sb * NF_TILE], f16, tag="e8")
            base = b * sb * NF_TILE
            nc.sync.dma_start(out=e8, in_=eps_d[:, base : base + sb * NF_TILE])
            esm = inp.tile([128, C, 3], f32, tag="esm")
            nc.sync.dma_start(
                out=esm,
                in_=epsm_d[:, b * C * 3 : (b + 1) * C * 3].rearrange(
                    "p (c d) -> p c d", d=3
                ),
            )
            W = wp.tile([128, C, 32], f32, tag="W")

            for j in range(sb):
                rhs0 = e8[:, j * NF_TILE : (j + 1) * NF_TILE]
                Pp = pps.tile([32, NF_TILE], f32, tag="Pp")
                for g in range(4):
                    h1p = h1ps.tile([128, NF_TILE], f32, tag="h1p")
                    nc.tensor.matmul(
                        h1p, lhsT=wL1_s[:, g, :], rhs=rhs0[0:2, :],
                        start=True, stop=True,
                    )
                    h1 = mlp.tile([128, NF_TILE], f16, tag="h1")
                    if g < 2:       # relu1: 2 groups on DVE, 2 on ACT
                        nc.vector.tensor_scalar(
                            out=h1, in0=h1p,
                            scalar1=b1_s[:, g : g + 1], scalar2=0.0,
                            op0=Alu.add, op1=Alu.max,
                        )
                    else:
                        nc.scalar.activation(
                            h1, h1p, Act.Relu, bias=b1_s[:, g : g + 1], scale=1.0
                        )
                    h2p = h2ps.tile([128, NF_TILE], f32, tag="h2p")
                    nc.tensor.matmul(
                        h2p, lhsT=wL2_s[:, g, :], rhs=h1, start=True, stop=True,
                    )
                    h2 = mlp.tile([128, NF_TILE], f16, tag="h2", bufs=6)
                    if g < 2:       # relu2: complementary split
                        nc.scalar.activation(
                            h2, h2p, Act.Relu, bias=b2_s[:, g : g + 1], scale=1.0
                        )
                    else:
                        nc.vector.tensor_scalar(
                            out=h2, in0=h2p,
                            scalar1=b2_s[:, g : g + 1], scalar2=0.0,
                            op0=Alu.add, op1=Alu.max,
                        )
                    nc.tensor.matmul(
                        Pp, lhsT=wL3_s[:, g, :], rhs=h2,
                        start=(g == 0), stop=(g == 3), skip_group_check=True,
                    )
                P_sb = psb.tile([32, NF_TILE], f16, tag="P_sb")
                nc.scalar.activation(
                    P_sb, Pp, Act.Identity, bias=b3z_s[:, 0:1], scale=1.0
                )
                tp = tps.tile([128, CHUNKS * 32], f16, tag="tp")
                for c in range(CHUNKS):
                    nc.tensor.transpose(
                        tp[:, 32 * c : 32 * (c + 1)],
                        P_sb[:, 128 * c : 128 * (c + 1)],
                        id16,
                    )
                nc.vector.tensor_copy(
                    out=W[:, j * CHUNKS : (j + 1) * CHUNKS, :],
                    in_=tp.rearrange("p (c r) -> p c r", r=32),
                )

            # ---- sample-major combine for this batch ----
            # z-init from the sample-major eps copy (ACT scale+bias, DVE acc)
            zc = cmb.tile([128, C, 2], f32, tag="zc")
            nc.scalar.activation(
                zc[:, :, 0:1], esm[:, :, 0:1], Act.Identity,
                bias=lcb[:, 3:4], scale=lcb[:, 2:3],
            )
            nc.vector.scalar_tensor_tensor(
                out=zc[:, :, 0:1], in0=esm[:, :, 1:2], scalar=lcb[:, 4:5],
                in1=zc[:, :, 0:1], op0=Alu.mult, op1=Alu.add,
            )
            nc.scalar.activation(
                zc[:, :, 1:2], esm[:, :, 0:1], Act.Identity,
                bias=lcb[:, 6:7], scale=lcb[:, 5:6],
            )
            nc.vector.scalar_tensor_tensor(
                out=zc[:, :, 1:2], in0=esm[:, :, 1:2], scalar=lcb[:, 7:8],
                in1=zc[:, :, 1:2], op0=Alu.mult, op1=Alu.add,
            )
            nc.vector.scalar_tensor_tensor(
                out=zc[:, :, 1:2], in0=esm[:, :, 2:3], scalar=lcb[:, 8:9],
                in1=zc[:, :, 1:2], op0=Alu.mult, op1=Alu.add,
            )
            # W block cols: 8g+2m+cc = s(flow 2g+m, comp cc); +4 = t
            Wg = W.rearrange("p c (g j) -> p c g j", j=8)
            St = cmb.tile([128, C, 16], f32, tag="St")
            nc.scalar.activation(
                St.rearrange("p c (g j) -> p c g j", j=4),
                Wg[:, :, :, 0:4],
                Act.Tanh,
            )
            EX = cmb.tile([128, C, 2, 8], f32, tag="EX")
            nc.gpsimd.memset(EX[:, :, :, 7:8], 0.0)
            for f in range(6, -1, -1):
                g1, m1 = divmod(f + 1, 2)
                o = 4 * g1 + 2 * m1
                nc.gpsimd.tensor_add(
                    out=EX[:, :, :, f],
                    in0=EX[:, :, :, f + 1],
                    in1=St[:, :, o : o + 2],
                )
            AT = cmb.tile([128, C, 2], f32, tag="AT")
            nc.gpsimd.tensor_add(out=AT, in0=EX[:, :, :, 0], in1=St[:, :, 0:2])
            ldv = cmb.tile([128, C], f32, tag="ldv")
            nc.gpsimd.tensor_add(out=ldv, in0=AT[:, :, 0], in1=AT[:, :, 1])
            nc.scalar.activation(EX, EX, Act.Exp)
            nc.scalar.activation(AT, AT, Act.Exp)
            TM = cmb.tile([128, C, 2, 8], f32, tag="TM")
            for cc in range(2):
                nc.gpsimd.tensor_mul(
                    out=TM[:, :, cc, :].rearrange("p c (g m) -> p c g m", m=2),
                    in0=Wg[:, :, :, 4 + cc : 8 : 2],
                    in1=EX[:, :, cc, :].rearrange("p c (g m) -> p c g m", m=2),
                )
            Tt = cmb.tile([128, C, 2], f32, tag="Tt")
            nc.vector.reduce_sum(out=Tt, in_=TM, axis=mybir.AxisListType.X)
            ZP = cmb.tile([128, C, 2], f32, tag="ZP")
            nc.gpsimd.tensor_mul(out=ZP, in0=zc, in1=AT)
            nc.gpsimd.tensor_add(out=ZP, in0=ZP, in1=Tt)
            pst = cmb.tile([128, 2], f32, tag="pst")
            nc.vector.reduce_sum(
                out=pst, in_=ZP.rearrange("p c d -> p d c"), axis=mybir.AxisListType.X
            )
            nc.gpsimd.tensor_add(out=psacc, in0=psacc, in1=pst)
            O = cmb.tile([128, C, 3], f32, tag="O")
            nc.scalar.activation(O[:, :, 1:3], ZP, Act.Exp)
            nc.scalar.activation(
                O[:, :, 0:1], esm[:, :, 0:1], Act.Identity,
                bias=lcb[:, 1:2], scale=lcb[:, 0:1],
            )
            nc.sync.dma_start(
                out=z_d[b * C : (b + 1) * C, :, :].rearrange("c d p -> p c d"),
                in_=O,
            )
            nc.sync.dma_start(
                out=ld_d[b * C : (b + 1) * C, :].rearrange("c p -> p c"),
                in_=ldv,
            )
        nc.sync.dma_start(out=ps_d[:, :], in_=psacc)

    if legalize:
        _legalize_waits(nc, mybir)
    return nc


def _legalize_waits(nc, mybir):
    """walrus in this container fits only ONE sync-wait per instruction
    struct; hoist extra waits onto engine NoOps inserted just before."""
    k = 0
    for fn in nc.m.functions:
        for blk in fn.blocks:
            out = []
            for inst in blk.instructions:
                si = inst.sync_info
                waits = list(si.on_wait) if si and si.on_wait else []
                if len(waits) > 1:
                    for w in waits[:-1]:
                        k += 1
                        nop = mybir.InstNoOp(name=f"nopw-{k}", ins=[], outs=[])
                        nop.engine = inst.engine
                        nop.sync_info = mybir.SyncInfo(on_wait=[w], on_update=[])
                        out.append(nop)
                    inst.sync_info = mybir.SyncInfo(
                        on_wait=[waits[-1]], on_update=list(si.on_update or [])
                    )
                out.append(inst)
            if k:
                blk.instructions = out


def _get_nc(nc_samp=NC_SAMP, sb=8):
    key = (nc_samp, sb)
    if key not in _CACHE:
        _CACHE[key] = _build_bass(nc_samp, sb)
    return _CACHE[key]


TRACE = False          # set by test harness to capture an NTFF profile
LAST_RESULT = None


def kernel(L_tril, base_mean, W1, b1, W2, b2, W3, b3, eps):
    global LAST_RESULT
    from concourse.bass_utils import run_bass_kernel_spmd

    nc = _get_nc()
    wts = _pack_weights(L_tril, base_mean, W1, b1, W2, b2, W3, b3)
    eps = np.ascontiguousarray(eps, dtype=np.float32)

    epsT = eps.T                                      # [3, N] fp32 view
    e_h = epsT.astype(np.float16)
    e_l = (epsT - e_h.astype(np.float32)).astype(np.float16)
    eps9 = np.empty((9, N), np.float16)
    eps9[0], eps9[2], eps9[4] = e_h[0], e_h[1], e_h[2]
    eps9[1], eps9[3], eps9[5] = e_l[0], e_l[1], e_l[2]
    eps9[6], eps9[7], eps9[8] = e_h[0], e_h[1], e_h[2]

    in_maps = []
    nt = NC_SAMP // 128
    for k in range(NCORES):
        shard = eps[k * NC_SAMP : (k + 1) * NC_SAMP]
        epsm = np.ascontiguousarray(
            shard.reshape(nt, 128, DIM).transpose(1, 0, 2).reshape(128, nt * DIM)
        )
        m = {
            "eps": np.ascontiguousarray(eps9[:, k * NC_SAMP : (k + 1) * NC_SAMP]),
            "epsm": epsm,
        }
        m.update(wts)
        in_maps.append(m)

    res = run_bass_kernel_spmd(
        nc, in_maps, core_ids=list(range(NCORES)), trace=TRACE
    )
    LAST_RESULT = res

    z_full = np.empty((N, DIM), np.float32)
    ld_full = np.empty((N,), np.float32)
    gsum = 0.0
    for k, r in enumerate(res.results):
        zb = r["z_out"]                                   # [NT, 3, 128]
        z_full[k * NC_SAMP : (k + 1) * NC_SAMP] = (
            zb.transpose(0, 2, 1).reshape(NC_SAMP, DIM)
        )
        ld_full[k * NC_SAMP : (k + 1) * NC_SAMP] = r["ld_out"].reshape(NC_SAMP)
        gsum += r["ps_out"].sum(dtype=np.float64)
    ld_full = ld_full + np.float32(gsum)
    return z_full, ld_full


# revision 62
# speedup vs baseline: 1.1278x; 1.1278x over previous
"""Trainium2 Bass kernel for nn_NormalizingFlow (dense_mlp, 8 flows, H=64).

Math (validated against the jax reference to ~7e-7 rel err):
  Because SPLIT=1, z[:,0] is invariant across all 8 coupling flows, so every
  flow's MLP reads the same scalar z0 = m0 + Lc00*e0.  The affine flow
  composition collapses:
    s_f = tanh(p_f[:2]),  t_f = p_f[2:]          (p_f = MLP_f(z0))
    z2_final = z2_init * exp(sum_f s_f) + sum_f t_f * exp(sum_{g>f} s_g)
    ld       = sum_{f,c} s_{f,c}
  Outputs: z = [z0, exp(z1p), exp(z2p)], ld_total = ld + sum_all(z1p)+sum_all(z2p).

Layout strategy (per core, data-parallel over 8 cores):
  * feature-major MLP: activations [feat_part, sample_free], 2 flows packed
    per 128-partition group (4 groups), block-diag W2 in one K=128 matmul.
  * per 512-sample supertile: L1 (K=1), relu(DVE,+bias), L2 (K=128),
    relu(ACT,+bias), L3 (K=128,M=8) + z-init (K=3,M=3) -> P psum [35,512],
    bias-add copy (ACT) -> SBUF, PE-transpose per 128-chunk -> sample-major.
  * per 8-supertile batch: tanh/suffix-sum/exp/combine in sample-major
    [128, 32chunk, ...] tiles (full-lane DVE/ACT, few instructions).
"""

import numpy as np

N = 1048576
DIM = 3
H = 64
NF = 8
NCORES = 8
NC_SAMP = N // NCORES        # 131072 samples per core
NF_TILE = 512                # samples per supertile (PSUM fp32 bank limit)
CHUNKS = NF_TILE // 128      # 4

_CACHE = {}

# fp32 constant pack (identity for PE transposes + engine biases)
OFF_ID = 0              # [128, 128] identity
OFF_B1 = 128            # [128, 4] relu1 bias
OFF_B2 = 132            # [128, 4] relu2 bias
OFF_B3 = 136            # [32, 1] p bias (b3)
OFF_LC = 137            # [128, 9] replicated [Lc00,m0,Lc10,m1,Lc11,Lc20,m2,Lc21,Lc22]
WPACK_COLS = 146
# fp16 matmul-weight pack
OFF_L2 = 0              # 4 x [128, 128] L2 block-diag lhsT
OFF_L3 = 512            # 4 x [128, 32] L3 lhsT (col-tiled, M=32 strips)
OFF_L1 = 652            # 4 x [2, 128] L1 lhsT (partitions 0-1: [w1; w1])
OFF_LZ = 1164           # [32, 32] fp16 identity for the P transposes
WPACK16_COLS = 1200


def _pack_weights(L_tril, base_mean, W1, b1, W2, b2, W3, b3):
    """Host-side prep of the tiny parameter tensors (fp32)."""
    f32 = np.float32
    L = np.tril(L_tril).astype(f32) + np.eye(DIM, dtype=f32) * f32(1e-6)
    cov = (L @ L.T).astype(f32)
    Lc = np.linalg.cholesky(cov.astype(np.float64)).astype(f32)

    W1c = W1[:, :, 0].astype(f32)                      # [8, 64]
    W1p = (W1c * Lc[0, 0]).astype(f32)
    b1p = (b1 + W1c * base_mean[0]).astype(f32)

    f16 = np.float16
    wpack = np.zeros((128, WPACK_COLS), f32)
    wpk16 = np.zeros((128, WPACK16_COLS), f16)
    for g in range(4):
        fa, fb = 2 * g, 2 * g + 1
        # L2 block-diag lhsT: [:, g*128:(g+1)*128]
        wpk16[:64, OFF_L2 + 128 * g : OFF_L2 + 128 * g + 64] = W2[fa].T
        wpk16[64:, OFF_L2 + 128 * g + 64 : OFF_L2 + 128 * (g + 1)] = W2[fb].T
        # L1 lhsT [2, 128] (rhs rows are [e0_hi; e0_lo])
        wrow = np.concatenate([W1p[fa], W1p[fb]]).astype(f16)
        wpk16[0, OFF_L1 + 128 * g : OFF_L1 + 128 * (g + 1)] = wrow
        wpk16[1, OFF_L1 + 128 * g : OFF_L1 + 128 * (g + 1)] = wrow
        # L3 lhsT [128, 32] per group: filled cols 8g+j (P row 8g+j,
        # j = 2m+c for s, 4+2m+c for t, flow 2g+m); all 4 groups form one
        # M=32 PSUM accumulation group (zeros elsewhere).
        o = OFF_L3 + 32 * g + 8 * g
        wpk16[:64, o + 0] = W3[fa][0]
        wpk16[:64, o + 1] = W3[fa][1]
        wpk16[64:, o + 2] = W3[fb][0]
        wpk16[64:, o + 3] = W3[fb][1]
        wpk16[:64, o + 4] = W3[fa][2]
        wpk16[:64, o + 5] = W3[fa][3]
        wpk16[64:, o + 6] = W3[fb][2]
        wpk16[64:, o + 7] = W3[fb][3]
        wpack[:64, OFF_B1 + g] = b1p[fa]
        wpack[64:, OFF_B1 + g] = b1p[fb]
        wpack[:64, OFF_B2 + g] = b2[fa]
        wpack[64:, OFF_B2 + g] = b2[fb]
        wpack[8 * g + 0, OFF_B3] = b3[fa][0]
        wpack[8 * g + 1, OFF_B3] = b3[fa][1]
        wpack[8 * g + 2, OFF_B3] = b3[fb][0]
        wpack[8 * g + 3, OFF_B3] = b3[fb][1]
        wpack[8 * g + 4, OFF_B3] = b3[fa][2]
        wpack[8 * g + 5, OFF_B3] = b3[fa][3]
        wpack[8 * g + 6, OFF_B3] = b3[fb][2]
        wpack[8 * g + 7, OFF_B3] = b3[fb][3]
    wpack[:, OFF_ID : OFF_ID + 128] = np.eye(128, dtype=f32)
    wpk16[0:32, OFF_LZ : OFF_LZ + 32] = np.eye(32, dtype=f16)
    # replicated Lc/mean scalars for the gpsimd sample-major z-init
    lcv = [Lc[0, 0], base_mean[0], Lc[1, 0], base_mean[1], Lc[1, 1],
           Lc[2, 0], base_mean[2], Lc[2, 1], Lc[2, 2]]
    wpack[:, OFF_LC : OFF_LC + 9] = np.asarray(lcv, f32)[None, :]
    return {"wpack": wpack, "wpk16": wpk16}


def _build_bass(nc_samp=NC_SAMP, sb=8, legalize=True):
    """Build the Bass program for one core processing nc_samp samples."""
    from contextlib import ExitStack

    import concourse.bass as bass
    import concourse.tile as tile
    from concourse import mybir

    f32 = mybir.dt.float32
    Alu = mybir.AluOpType
    Act = mybir.ActivationFunctionType

    f16 = mybir.dt.float16
    st_total = nc_samp // NF_TILE          # supertiles
    nb = st_total // sb                    # batches
    assert st_total % sb == 0
    C = sb * CHUNKS                        # chunks per batch
    NT = nc_samp // 128                    # total chunks

    nc = bass.Bass(trn_type="TRN2")

    # eps arrives pre-transposed + fp16 hi/lo split (host prep):
    # rows [e0h, e0l, e1h, e1l, e2h, e2l, e0h, e1h, e2h]
    eps_d = nc.dram_tensor("eps", [9, nc_samp], f16, kind="ExternalInput")
    epsm_d = nc.dram_tensor("epsm", [128, NT * 3], f32, kind="ExternalInput")
    wpk_d = nc.dram_tensor("wpack", [128, WPACK_COLS], f32, kind="ExternalInput")
    w16_d = nc.dram_tensor("wpk16", [128, WPACK16_COLS], f16, kind="ExternalInput")

    z_d = nc.dram_tensor("z_out", [NT, 3, 128], f32, kind="ExternalOutput")
    ld_d = nc.dram_tensor("ld_out", [NT, 128], f32, kind="ExternalOutput")
    ps_d = nc.dram_tensor("ps_out", [128, 2], f32, kind="ExternalOutput")

    with ExitStack() as ctx:
        tc = ctx.enter_context(tile.TileContext(nc))
        singles = ctx.enter_context(tc.tile_pool(name="singles", bufs=1))
        inp = ctx.enter_context(tc.tile_pool(name="inp", bufs=2))
        mlp = ctx.enter_context(tc.tile_pool(name="mlp", bufs=3))
        psb = ctx.enter_context(tc.tile_pool(name="psb", bufs=2))
        wp = ctx.enter_context(tc.tile_pool(name="wp", bufs=2))
        cmb = ctx.enter_context(tc.tile_pool(name="cmb", bufs=2))
        h1ps = ctx.enter_context(tc.tile_pool(name="h1ps", bufs=2, space="PSUM"))
        h2ps = ctx.enter_context(tc.tile_pool(name="h2ps", bufs=2, space="PSUM"))
        pps = ctx.enter_context(tc.tile_pool(name="pps", bufs=2, space="PSUM"))
        tps = ctx.enter_context(tc.tile_pool(name="tps", bufs=2, space="PSUM"))

        # ---- constants: one DMA per pack + per-engine absorber ops so no
        # matmul/engine op ever needs more than one sync wait (the walrus
        # here fits a single wait per instruction struct). ----
        wpk = singles.tile([128, WPACK_COLS], f32)
        w16 = singles.tile([128, WPACK16_COLS], f16)
        psacc = singles.tile([128, 2], f32)
        scr_v = singles.tile([128, 18], f32)
        scr_a = singles.tile([128, 14], f32)
        nc.sync.dma_start(out=wpk, in_=wpk_d[:, :])
        nc.sync.dma_start(out=w16, in_=w16_d[:, :])
        # PE absorbers: fp16 standalone LDWEIGHTS over the whole fp16 pack,
        # and a dummy fp32 transpose touching the identity region.
        for c0 in range(0, WPACK16_COLS, 128):
            nc.tensor.ldweights(w16[:, c0 : min(c0 + 128, WPACK16_COLS)])
        dpt = tps.tile([1, 2], f32, tag="tp")
        nc.tensor.transpose(
            dpt[0:1, 0:1], wpk[0:1, OFF_ID : OFF_ID + 1],
            wpk[0:1, OFF_ID : OFF_ID + 1],
        )
        nc.vector.tensor_copy(out=scr_v, in_=wpk[:, OFF_B1 : OFF_LC + 9])
        nc.scalar.copy(out=scr_a, in_=wpk[:, OFF_B2 : OFF_LC + 9])
        nc.vector.memset(psacc, 0.0)

        wL2_s = w16[:, OFF_L2 : OFF_L2 + 512].rearrange("p (g m) -> p g m", m=128)
        wL3_s = w16[:, OFF_L3 : OFF_L3 + 128].rearrange("p (g m) -> p g m", m=32)
        wL1_s = w16[0:2, OFF_L1 : OFF_L1 + 512].rearrange("p (g m) -> p g m", m=128)
        b1_s = wpk[:, OFF_B1 : OFF_B1 + 4]
        b2_s = wpk[:, OFF_B2 : OFF_B2 + 4]
        b3z_s = wpk[0:32, OFF_B3 : OFF_B3 + 1]
        id_s = wpk[:, OFF_ID : OFF_ID + 128]
        id16 = w16[0:32, OFF_LZ : OFF_LZ + 32]
        lcb = wpk[:, OFF_LC : OFF_LC + 9]

        for b in range(nb):
            e8 = inp.tile([9, sb * NF_TILE], f16, tag="e8")
            base = b * sb * NF_TILE
            nc.sync.dma_start(out=e8, in_=eps_d[:, base : base + sb * NF_TILE])
            esm = inp.tile([128, C, 3], f32, tag="esm")
            nc.sync.dma_start(
                out=esm,
                in_=epsm_d[:, b * C * 3 : (b + 1) * C * 3].rearrange(
                    "p (c d) -> p c d", d=3
                ),
            )
            W = wp.tile([128, C, 32], f32, tag="W")

            for j in range(sb):
                rhs0 = e8[:, j * NF_TILE : (j + 1) * NF_TILE]
                Pp = pps.tile([32, NF_TILE], f32, tag="Pp")
                for g in range(4):
                    h1p = h1ps.tile([128, NF_TILE], f32, tag="h1p")
                    nc.tensor.matmul(
                        h1p, lhsT=wL1_s[:, g, :], rhs=rhs0[0:2, :],
                        start=True, stop=True,
                    )
                    h1 = mlp.tile([128, NF_TILE], f16, tag="h1")
                    if g < 2:       # relu1: 2 groups on DVE, 2 on ACT
                        nc.vector.tensor_scalar(
                            out=h1, in0=h1p,
                            scalar1=b1_s[:, g : g + 1], scalar2=0.0,
                            op0=Alu.add, op1=Alu.max,
                        )
                    else:
                        nc.scalar.activation(
                            h1, h1p, Act.Relu, bias=b1_s[:, g : g + 1], scale=1.0
                        )
                    h2p = h2ps.tile([128, NF_TILE], f32, tag="h2p")
                    nc.tensor.matmul(
                        h2p, lhsT=wL2_s[:, g, :], rhs=h1, start=True, stop=True,
                    )
                    h2 = mlp.tile([128, NF_TILE], f16, tag="h2", bufs=6)
                    if g < 2:       # relu2: complementary split
                        nc.scalar.activation(
                            h2, h2p, Act.Relu, bias=b2_s[:, g : g + 1], scale=1.0
                        )
                    else:
                        nc.vector.tensor_scalar(
                            out=h2, in0=h2p,
                            scalar1=b2_s[:, g : g + 1], scalar2=0.0,
                            op0=Alu.add, op1=Alu.max,
                        )
                    nc.tensor.matmul(
                        Pp, lhsT=wL3_s[:, g, :], rhs=h2,
                        start=(g == 0), stop=(g == 3), skip_group_check=True,
                    )
                P_sb = psb.tile([32, NF_TILE], f16, tag="P_sb")
                nc.scalar.activation(
                    P_sb, Pp, Act.Identity, bias=b3z_s[:, 0:1], scale=1.0
                )
                tp = tps.tile([128, CHUNKS * 32], f16, tag="tp")
                for c in range(CHUNKS):
                    nc.tensor.transpose(
                        tp[:, 32 * c : 32 * (c + 1)],
                        P_sb[:, 128 * c : 128 * (c + 1)],
                        id16,
                    )
                nc.vector.tensor_copy(
                    out=W[:, j * CHUNKS : (j + 1) * CHUNKS, :],
                    in_=tp.rearrange("p (c r) -> p c r", r=32),
                )

            # ---- sample-major combine for this batch ----
            # z-init from the sample-major eps copy (ACT scale+bias, DVE acc)
            zc = cmb.tile([128, C, 2], f32, tag="zc")
            nc.scalar.activation(
                zc[:, :, 0:1], esm[:, :, 0:1], Act.Identity,
                bias=lcb[:, 3:4], scale=lcb[:, 2:3],
            )
            nc.vector.scalar_tensor_tensor(
                out=zc[:, :, 0:1], in0=esm[:, :, 1:2], scalar=lcb[:, 4:5],
                in1=zc[:, :, 0:1], op0=Alu.mult, op1=Alu.add,
            )
            nc.scalar.activation(
                zc[:, :, 1:2], esm[:, :, 0:1], Act.Identity,
                bias=lcb[:, 6:7], scale=lcb[:, 5:6],
            )
            nc.vector.scalar_tensor_tensor(
                out=zc[:, :, 1:2], in0=esm[:, :, 1:2], scalar=lcb[:, 7:8],
                in1=zc[:, :, 1:2], op0=Alu.mult, op1=Alu.add,
            )
            nc.vector.scalar_tensor_tensor(
                out=zc[:, :, 1:2], in0=esm[:, :, 2:3], scalar=lcb[:, 8:9],
                in1=zc[:, :, 1:2], op0=Alu.mult, op1=Alu.add,
            )
            # W block cols: 8g+2m+cc = s(flow 2g+m, comp cc); +4 = t
            Wg = W.rearrange("p c (g j) -> p c g j", j=8)
            St = cmb.tile([128, C, 16], f32, tag="St")
            nc.scalar.activation(
                St.rearrange("p c (g j) -> p c g j", j=4),
                Wg[:, :, :, 0:4],
                Act.Tanh,
            )
            EX = cmb.tile([128, C, 2, 8], f32, tag="EX")
            nc.gpsimd.memset(EX[:, :, :, 7:8], 0.0)
            for f in range(6, -1, -1):
                g1, m1 = divmod(f + 1, 2)
                o = 4 * g1 + 2 * m1
                nc.gpsimd.tensor_add(
                    out=EX[:, :, :, f],
                    in0=EX[:, :, :, f + 1],
                    in1=St[:, :, o : o + 2],
                )
            AT = cmb.tile([128, C, 2], f32, tag="AT")
            nc.gpsimd.tensor_add(out=AT, in0=EX[:, :, :, 0], in1=St[:, :, 0:2])
            ldv = cmb.tile([128, C], f32, tag="ldv")
            nc.gpsimd.tensor_add(out=ldv, in0=AT[:, :, 0], in1=AT[:, :, 1])
            nc.scalar.activation(EX, EX, Act.Exp)
            nc.scalar.activation(AT, AT, Act.Exp)
            TM = cmb.tile([128, C, 2, 8], f32, tag="TM")
            for cc in range(2):
                nc.gpsimd.tensor_mul(
                    out=TM[:, :, cc, :].rearrange("p c (g m) -> p c g m", m=2),
                    in0=Wg[:, :, :, 4 + cc : 8 : 2],
                    in1=EX[:, :, cc, :].rearrange("p c (g m) -> p c g m", m=2),
                )
            Tt = cmb.tile([128, C, 2], f32, tag="Tt")
            nc.vector.reduce_sum(out=Tt, in_=TM, axis=mybir.AxisListType.X)
            ZP = cmb.tile([128, C, 2], f32, tag="ZP")
            nc.gpsimd.tensor_mul(out=ZP, in0=zc, in1=AT)
            nc.gpsimd.tensor_add(out=ZP, in0=ZP, in1=Tt)
            pst = cmb.tile([128, 2], f32, tag="pst")
            nc.vector.reduce_sum(
                out=pst, in_=ZP.rearrange("p c d -> p d c"), axis=mybir.AxisListType.X
            )
            nc.gpsimd.tensor_add(out=psacc, in0=psacc, in1=pst)
            O = cmb.tile([128, C, 3], f32, tag="O")
            nc.scalar.activation(O[:, :, 1:3], ZP, Act.Exp)
            nc.scalar.activation(
                O[:, :, 0:1], esm[:, :, 0:1], Act.Identity,
                bias=lcb[:, 1:2], scale=lcb[:, 0:1],
            )
            nc.sync.dma_start(
                out=z_d[b * C : (b + 1) * C, :, :].rearrange("c d p -> p c d"),
                in_=O,
            )
            nc.sync.dma_start(
                out=ld_d[b * C : (b + 1) * C, :].rearrange("c p -> p c"),
                in_=ldv,
            )
        nc.sync.dma_start(out=ps_d[:, :], in_=psacc)

    if legalize:
        _legalize_waits(nc, mybir)
    return nc


def _legalize_waits(nc, mybir):
    """walrus in this container fits only ONE sync-wait per instruction
    struct; hoist extra waits onto engine NoOps inserted just before."""
    k = 0
    for fn in nc.m.functions:
        for blk in fn.blocks:
            out = []
            for inst in blk.instructions:
                si = inst.sync_info
                waits = list(si.on_wait) if si and si.on_wait else []
                if len(waits) > 1:
                    for w in waits[:-1]:
                        k += 1
                        nop = mybir.InstNoOp(name=f"nopw-{k}", ins=[], outs=[])
                        nop.engine = inst.engine
                        nop.sync_info = mybir.SyncInfo(on_wait=[w], on_update=[])
                        out.append(nop)
                    inst.sync_info = mybir.SyncInfo(
                        on_wait=[waits[-1]], on_update=list(si.on_update or [])
                    )
                out.append(inst)
            if k:
                blk.instructions = out


def _get_nc(nc_samp=NC_SAMP, sb=8):
    key = (nc_samp, sb)
    if key not in _CACHE:
        _CACHE[key] = _build_bass(nc_samp, sb)
    return _CACHE[key]


TRACE = False          # set by test harness to capture an NTFF profile
LAST_RESULT = None


def kernel(L_tril, base_mean, W1, b1, W2, b2, W3, b3, eps):
    global LAST_RESULT
    from concourse.bass_utils import run_bass_kernel_spmd

    nc = _get_nc()
    wts = _pack_weights(L_tril, base_mean, W1, b1, W2, b2, W3, b3)
    eps = np.ascontiguousarray(eps, dtype=np.float32)

    epsT = eps.T                                      # [3, N] fp32 view
    e_h = epsT.astype(np.float16)
    e_l = (epsT - e_h.astype(np.float32)).astype(np.float16)
    eps9 = np.empty((9, N), np.float16)
    eps9[0], eps9[2], eps9[4] = e_h[0], e_h[1], e_h[2]
    eps9[1], eps9[3], eps9[5] = e_l[0], e_l[1], e_l[2]
    eps9[6], eps9[7], eps9[8] = e_h[0], e_h[1], e_h[2]

    in_maps = []
    nt = NC_SAMP // 128
    for k in range(NCORES):
        shard = eps[k * NC_SAMP : (k + 1) * NC_SAMP]
        epsm = np.ascontiguousarray(
            shard.reshape(nt, 128, DIM).transpose(1, 0, 2).reshape(128, nt * DIM)
        )
        m = {
            "eps": np.ascontiguousarray(eps9[:, k * NC_SAMP : (k + 1) * NC_SAMP]),
            "epsm": epsm,
        }
        m.update(wts)
        in_maps.append(m)

    res = run_bass_kernel_spmd(
        nc, in_maps, core_ids=list(range(NCORES)), trace=TRACE
    )
    LAST_RESULT = res

    z_full = np.empty((N, DIM), np.float32)
    ld_full = np.empty((N,), np.float32)
    gsum = 0.0
    for k, r in enumerate(res.results):
        zb = r["z_out"]                                   # [NT, 3, 128]
        z_full[k * NC_SAMP : (k + 1) * NC_SAMP] = (
            zb.transpose(0, 2, 1).reshape(NC_SAMP, DIM)
        )
        ld_full[k * NC_SAMP : (k + 1) * NC_SAMP] = r["ld_out"].reshape(NC_SAMP)
        gsum += r["ps_out"].sum(dtype=np.float64)
    ld_full = ld_full + np.float32(gsum)
    return z_full, ld_full
